# revision 50
# baseline (speedup 1.0000x reference)
"""Trainium2 Bass kernel for nn_MultiHeadAttention (B=2, S=2048, E=1024, H=16).

Sharding: 8 NeuronCores = data-parallel over the 2 batches x tensor-parallel
over the 16 heads in 4 groups of 4 heads (Wq/Wk/Wv split column-wise, Wo
row-wise).  Each core computes a full-[S, E] partial of its batch's output;
the host sums the 4 head-group partials per batch.

Per-core pipeline (the ACT exp stream is the pace-setter at ~133us; all
other work is scheduled into 128 global "slots", one per exp op):
  Q.T/K.T[n, s]  per 512-col window: psX-accumulated e-outer matmuls
                 chasing column-block DMAs (first exp ~15us).
  S.T_h[k, q]    [64,128]x[64,512] per (window, head-pair, k-chunk),
                 head pair packed in one [128,1024] PSUM tile.
  P.T            one ACT exp op per (window, pair, k-chunk); fp16 out.
  O[q, d|sum]    TRANSPOSED P.V: stationary = P.T chunk [128k,128q],
                 moving = [V_h | ones] [128k, 65] -> accumulates
                 [128q, 65] in sub-bank PSUM slices (2x fewer PE rows
                 than the O.T orientation).  Rolls ~10 slots behind the
                 exp stream, catching up to lag ~2 by the last pair.
  normalize      DVE reciprocal of col 64 + per-partition scalar mult.
  O.T            crossbar DMA transpose [128q,128hd] -> [128hd,128q]
                 in-stream; PE transpose via identity in the tail.
  out[m, :]      stationary oT chunk [128 hd, 128 m] x moving Wo.T
                 [128 hd, 512 e], 2-chunk contraction, DVE copy + SWDGE
                 store; the last window stores fp16 halves (out16) on
                 the idle SP/ACT queues and the host upcasts.

dtypes: all matmul operands fp16 (1 cyc/row on PE); PSUM accumulation fp32;
softmax exp on fp32 scores.  Final rel err vs fp64 reference ~8e-4.
"""

import numpy as np
from contextlib import ExitStack

import concourse.bass as bass
import concourse.mybir as mybir
import concourse.tile as tile
from concourse.tile import ScopedClock
from concourse.bass_utils import run_bass_kernel_spmd

# ---------------------------------------------------------------------------
# Workarounds for the walrus build on this stack, which rejects more than ONE
# semaphore wait per instruction ("Too many sync wait commands").
# ---------------------------------------------------------------------------
_orig_commit_instruction = tile.TileContext._commit_instruction


def _commit_instruction(self, inst, lazy_reg_writes=True):
    si = getattr(inst, "sync_info", None)
    if si is not None and si.on_wait and len(si.on_wait) > 1:
        waits = list(si.on_wait)
        for w in waits[:-1]:
            nop = mybir.InstNoOp(
                name=self.nc.get_next_instruction_name(),
                ins=[], outs=[], engine=inst.engine,
            )
            nop.bass_nofuse = True
            nop.sync_info = mybir.SyncInfo(on_wait=[w], on_update=[])
            _orig_commit_instruction(self, nop, lazy_reg_writes=False)
        inst.sync_info = mybir.SyncInfo(
            on_wait=[waits[-1]], on_update=list(si.on_update or [])
        )
    return _orig_commit_instruction(self, inst, lazy_reg_writes)


def _drain_and_barrier(self, tick_clock, wait_clock):
    nc = self.nc
    drain_inst = nc.sync.drain()
    wait_clock.add_sem_waits(
        drain_inst.ins, ScopedClock({None: tick_clock.global_clock})
    )
    si = drain_inst.ins.sync_info
    waits = list(si.on_wait) if si and si.on_wait else []
    if len(waits) > 1:
        drain_inst.ins.sync_info = mybir.SyncInfo(
            on_wait=waits[:1], on_update=list(si.on_update or [])
        )
        for w in waits[1:]:
            extra = nc.sync.drain()
            esi = extra.ins.sync_info
            extra.ins.sync_info = mybir.SyncInfo(
                on_wait=[w],
                on_update=list(esi.on_update or []) if esi else [],
            )
    nc.all_engine_barrier()
    assert self.sems is not None
    popped = nc._tile_sem_poison_stack.pop()
    assert popped is self._sem_poison
    nc.clear_and_free_semaphores(list(self.sems.allocated().values()))
    nc.all_engine_barrier()


def _apply_tilefix():
    tile.TileContext._commit_instruction = _commit_instruction
    tile.TileContext._drain_and_barrier = _drain_and_barrier


_apply_tilefix()

# ---------------------------------------------------------------------------
# Problem constants (hardcoded)
# ---------------------------------------------------------------------------
B, S, E, H = 2, 2048, 1024, 16
HC, D = 4, 64              # heads per core, head dim
NCORES = 8
NE = E // 128              # 8  e-chunks
NW = S // 512              # 4  q/k windows
NK = S // 128              # 16 k-chunks
NM = S // 128              # 16 m-chunks

F32 = mybir.dt.float32
FP16 = mybir.dt.float16


def build(ptbufs=18, cfg=None):
    cfg = dict({
        "kslots": (2, 3, 6, 7, 10, 11),
        "vslots": tuple(range(14, 30)),
        "q1slots": (29, 31),
        "q23slots": (40, 56, 72, 88),
        "oslots": True,
        "budget": 1500,
        "prolog_whole": True,
        "lags": (8, 8, 8, 8, 8, 6, 4, 2),
        "offk": (9, 13),
        "offp0": 4,
        "obufs": 3,
        "onbufs": 8,
    }, **(cfg or {}))
    KSLOTS = cfg.get("kslots", (1, 3, 5, 7, 9, 11))
    Q1SLOTS = cfg.get("q1slots", (17, 19))
    Q23SLOTS = cfg.get("q23slots", (49, 54, 81, 86))
    VSLOTS_CFG = cfg.get("vslots",
                         (12, 13, 14, 15, 22, 23, 24, 25,
                          32, 33, 34, 35, 36, 37, 38, 39))
    BUDGET_CFG = cfg.get("budget", 1010)
    OUTBUDGET = cfg.get("outbudget", 440)
    OFFK = cfg.get("offk", ())
    OFFP0 = cfg.get("offp0", 0)
    OFFMAP = cfg.get("offmap")
    if OFFMAP is not None:
        OFFMAP = frozenset(OFFMAP)
    nc = bass.Bass()
    xqT = nc.dram_tensor("xqT", [E, S], FP16, kind="ExternalInput")
    xkT = nc.dram_tensor("xkT", [E, S], FP16, kind="ExternalInput")
    xvT = nc.dram_tensor("xvT", [E, S], FP16, kind="ExternalInput")
    wqT = nc.dram_tensor("wqT", [E, 256], FP16, kind="ExternalInput")
    wkT = nc.dram_tensor("wkT", [E, 256], FP16, kind="ExternalInput")
    wvT = nc.dram_tensor("wvT", [E, 256], FP16, kind="ExternalInput")
    woT = nc.dram_tensor("woT", [256, E], FP16, kind="ExternalInput")
    out = nc.dram_tensor("out", [S, E], F32, kind="ExternalOutput")
    out16 = nc.dram_tensor("out16", [512, E], FP16, kind="ExternalOutput")
    identT = nc.dram_tensor("identT", [128, 128], FP16, kind="ExternalInput")

    with tile.TileContext(nc) as tc, ExitStack() as ctx:
        consts = ctx.enter_context(tc.tile_pool(name="consts", bufs=1))
        wpool = ctx.enter_context(tc.tile_pool(name="w", bufs=1))
        actpool = ctx.enter_context(tc.tile_pool(name="acts", bufs=1))
        xkpool = ctx.enter_context(tc.tile_pool(name="xk", bufs=3))
        xqpool = ctx.enter_context(tc.tile_pool(name="xq", bufs=3))
        xvpool = ctx.enter_context(tc.tile_pool(name="xv", bufs=3))
        ptpool = ctx.enter_context(tc.tile_pool(name="pT", bufs=ptbufs))
        onpool = ctx.enter_context(
            tc.tile_pool(name="on", bufs=cfg.get("onbufs", 4)))
        rpool = ctx.enter_context(
            tc.tile_pool(name="recip", bufs=cfg.get("rbufs", 8)))
        opool = ctx.enter_context(
            tc.tile_pool(name="outstage", bufs=cfg.get("obufs", 2)))
        ohpool = ctx.enter_context(
            tc.tile_pool(name="outhalf", bufs=cfg.get("ohbufs", 8)))
        schpool = ctx.enter_context(
            tc.tile_pool(name="sch", bufs=cfg.get("schbufs", 2)))
        psS = ctx.enter_context(tc.tile_pool(name="psS", bufs=2, space="PSUM"))
        psOV = ctx.enter_context(tc.tile_pool(name="psOV", bufs=1, space="PSUM"))
        psX = ctx.enter_context(tc.tile_pool(name="psX", bufs=2, space="PSUM"))

        # preload the exp table before the hot loop
        dummy = consts.tile([1, 8], F32)
        nc.vector.memset(dummy[:], 0.0)
        nc.scalar.activation(dummy[:], dummy[:], mybir.ActivationFunctionType.Exp)

        wq_sb = wpool.tile([128, NE, 256], FP16)
        wk_sb = wpool.tile([128, NE, 256], FP16)
        wv_sb = wpool.tile([128, NE, 256], FP16)
        wo_sb = wpool.tile([128, 2, E], FP16)

        qT_sb = actpool.tile([128, 2, S], FP16)        # [(2 heads x d), pair, s]
        kT_sb = actpool.tile([128, 2, S], FP16)
        v_sb = actpool.tile([128, NK, HC, 65], FP16)   # [s%128, k, h, V_h|ones]
        oT_sb = actpool.tile([128, 2, S], FP16, name="oT")  # [(h2 d), pair, s]

        nc.vector.memset(v_sb[:, :, :, 64:65], 1.0)

        # ---- DMA emission order on the SP queue (arrival order == need) ---
        def colblock(x, j):
            return x[:, j * 512:(j + 1) * 512].rearrange(
                "(ec p) s -> p ec s", p=128)

        def halfblock(x, j, h):
            return x[h * 512:(h + 1) * 512,
                     j * 512:(j + 1) * 512].rearrange(
                "(ec p) s -> p ec s", p=128)

        xq_blks = {}
        xk_blks = {}
        xv_blks = {}

        def load_x(pool, src, blks, j, tag, split=False):
            t = pool.tile([128, NE, 512], FP16, tag=tag, name=f"{tag}{j}")
            if split:
                # two half-e DMAs so the projection can chase the first half
                nc.sync.dma_start(t[:, 0:4, :], halfblock(src, j, 0))
                nc.sync.dma_start(t[:, 4:8, :], halfblock(src, j, 1))
            else:
                nc.sync.dma_start(t[:], colblock(src, j))
            blks[j] = t

        # arrival order == need order (single serialized DMA device).
        # wk/wq land as nch halves and x window 0 as e-halves so the w0
        # projections chase the prolog DMA stream at fine grain.
        t0k = xkpool.tile([128, NE, 512], FP16, tag="xkb", name="xkb0")
        xk_blks[0] = t0k
        t0q = xqpool.tile([128, NE, 512], FP16, tag="xqb", name="xqb0")
        xq_blks[0] = t0q
        if cfg.get("prolog_whole"):
            nc.sync.dma_start(
                wk_sb[:], wkT.rearrange("(ec p) n -> p ec n", p=128))
            nc.sync.dma_start(t0k[:, 0:4, :], halfblock(xkT, 0, 0))
            nc.sync.dma_start(t0k[:, 4:8, :], halfblock(xkT, 0, 1))
            nc.sync.dma_start(
                wq_sb[:], wqT.rearrange("(ec p) n -> p ec n", p=128))
            nc.sync.dma_start(t0q[:, 0:4, :], halfblock(xqT, 0, 0))
            nc.sync.dma_start(t0q[:, 4:8, :], halfblock(xqT, 0, 1))
        else:
            nc.sync.dma_start(wk_sb[:, :, 0:128],
                              wkT[:, 0:128].rearrange("(ec p) n -> p ec n", p=128))
            nc.sync.dma_start(t0k[:, 0:4, :], halfblock(xkT, 0, 0))
            nc.sync.dma_start(wk_sb[:, :, 128:256],
                              wkT[:, 128:256].rearrange("(ec p) n -> p ec n", p=128))
            nc.sync.dma_start(t0k[:, 4:8, :], halfblock(xkT, 0, 1))
            nc.sync.dma_start(wq_sb[:, :, 0:128],
                              wqT[:, 0:128].rearrange("(ec p) n -> p ec n", p=128))
            nc.sync.dma_start(t0q[:, 0:4, :], halfblock(xqT, 0, 0))
            nc.sync.dma_start(wq_sb[:, :, 128:256],
                              wqT[:, 128:256].rearrange("(ec p) n -> p ec n", p=128))
            nc.sync.dma_start(t0q[:, 4:8, :], halfblock(xqT, 0, 1))
        for j in range(1, NW):
            load_x(xkpool, xkT, xk_blks, j, "xkb", split=True)
        nc.sync.dma_start(wv_sb[:], wvT.rearrange("(ec p) n -> p ec n", p=128))
        load_x(xvpool, xvT, xv_blks, 0, "xvb")
        load_x(xvpool, xvT, xv_blks, 1, "xvb")
        load_x(xqpool, xqT, xq_blks, 1, "xqb")
        load_x(xvpool, xvT, xv_blks, 2, "xvb")
        load_x(xvpool, xvT, xv_blks, 3, "xvb")
        load_x(xqpool, xqT, xq_blks, 2, "xqb")
        load_x(xqpool, xqT, xq_blks, 3, "xqb")
        nc.sync.dma_start(wo_sb[:], woT.rearrange("(j p) e -> p j e", p=128))
        id_sb = wpool.tile([128, 128], FP16)
        nc.sync.dma_start(id_sb[:], identT[:, :])

        # ---- building blocks -------------------------------------------
        def wproj(w_sb, blk, dst, win):
            """project one 512-col window of x into dst[:, :, win] (2 psX)."""
            ws = slice(win * 512, (win + 1) * 512)
            for nch in range(2):
                ps = psX.tile([128, 512], F32, tag="px", name=f"pj{win}_{nch}")
                for e in range(NE):
                    nc.tensor.matmul(
                        ps[:],
                        w_sb[:, e, nch * 128:(nch + 1) * 128],
                        blk[:, e, :],
                        start=(e == 0), stop=(e == NE - 1))
                nc.vector.tensor_copy(dst[:, nch, ws], ps[:])

        def vproj(m):
            blk = xv_blks[m // 4]
            ps = psX.tile([128, 512], F32, tag="px", name=f"vp{m}")
            for e in range(NE):
                nc.tensor.matmul(
                    ps[:, 0:256],
                    blk[:, e, (m % 4) * 128:(m % 4 + 1) * 128],
                    wv_sb[:, e, :],
                    start=(e == 0), stop=(e == NE - 1))
            nc.vector.tensor_copy(
                v_sb[:, m, :, 0:64],
                ps[:, 0:256].rearrange("p (h c) -> p h c", h=HC))

        def ov_group(ovts, pair, pT, kc):
            """8 transposed-PV matmuls for one k-chunk; sub-bank psum accum."""
            for t_i, (ovt, qlocs) in enumerate(ovts):
                for si, (h2, ql) in enumerate(
                        [(h, q) for h in range(2) for q in qlocs]):
                    nc.tensor.matmul(
                        ovt[:, ql % 2, h2, :],
                        pT[:, h2 * 512 + ql * 128: h2 * 512 + (ql + 1) * 128],
                        v_sb[:, kc, 2 * pair + h2, :],
                        start=(kc == 0 and si == 0),
                        stop=(kc == NK - 1 and si == 3),
                        skip_group_check=True)

        def finalize(ovts, w, pair):
            """normalize + crossbar-transpose one (window, pair)."""
            for ovt, qlocs in ovts:
                for ql in qlocs:
                    o_n = onpool.tile([128, 128], FP16, tag="on")
                    for h2 in range(2):
                        rt = rpool.tile([128, 1], F32, tag="rt")
                        nc.vector.reciprocal(rt[:], ovt[:, ql % 2, h2, 64:65])
                        nc.vector.tensor_scalar_mul(
                            o_n[:, h2 * 64:(h2 + 1) * 64],
                            ovt[:, ql % 2, h2, 0:64],
                            rt[:])
                    qs = slice(w * 512 + ql * 128, w * 512 + (ql + 1) * 128)
                    nc.sync.dma_start_transpose(oT_sb[:, pair, qs], o_n[:])

        def outproj_half(m, j, stage):
            ps = psX.tile([128, 512], F32, tag="px", name=f"op{m}_{j}")
            for jp in range(2):
                nc.tensor.matmul(
                    ps[:],
                    oT_sb[:, jp, m * 128:(m + 1) * 128],
                    wo_sb[:, jp, j * 512:(j + 1) * 512],
                    start=(jp == 0), stop=(jp == 1))
            nc.vector.tensor_copy(stage[:, j * 512:(j + 1) * 512], ps[:])
            if j == 1:
                nc.gpsimd.dma_start(out[m * 128:(m + 1) * 128, :], stage[:])

        # ---- global-slot schedule --------------------------------------
        # slot g = p*16 + kc carries scores(p,kc)+exp; OV work is lagged
        # OVLAG slots behind the exp stream (rolling across p boundaries).
        OVLAG = 10
        TAILOV = 8            # OV groups left for the post-stream tail
        from collections import defaultdict
        extras_pre = defaultdict(list)    # g -> thunks (feeders: proj work)
        extras_post = defaultdict(list)   # g -> thunks (drains: outproj)

        def sched_wproj(g, w_sb, blks, dst, win, nch=None):
            for n in ((0, 1) if nch is None else (nch,)):
                extras_pre[g].append(
                    lambda n=n, win=win: wproj1(w_sb, blks[win], dst, win, n))

        def wproj1(w_sb, blk, dst, win, nch, use_act=False):
            ws = slice(win * 512, (win + 1) * 512)
            ps = psX.tile([128, 512], F32, tag="px", name=f"pj{win}_{nch}")
            for e in range(NE):
                nc.tensor.matmul(
                    ps[:],
                    w_sb[:, e, nch * 128:(nch + 1) * 128],
                    blk[:, e, :],
                    start=(e == 0), stop=(e == NE - 1))
            if use_act:
                nc.scalar.copy(dst[:, nch, ws], ps[:])
            else:
                nc.vector.tensor_copy(dst[:, nch, ws], ps[:])

        # K windows 1-3 early in p0 (chasing the xk block DMAs)
        KSCHED = tuple(
            (g, 1 + i // 2, i % 2) for i, g in enumerate(cfg["kslots"]))
        for g, win, nch in KSCHED:
            sched_wproj(g, wk_sb, xk_blks, kT_sb, win, nch=nch)
        # V tiles: p0 is kproj-heavy, so vprojs go to p1/p2 (xv blocks have
        # all arrived by then); qproj(w1) interleaves in p1.
        VSLOTS = list(VSLOTS_CFG)
        for m in range(NM):
            extras_pre[VSLOTS[m]].append(lambda m=m: vproj(m))
        sched_wproj(Q1SLOTS[0], wq_sb, xq_blks, qT_sb, 1, nch=0)
        sched_wproj(Q1SLOTS[1], wq_sb, xq_blks, qT_sb, 1, nch=1)
        sched_wproj(Q23SLOTS[0], wq_sb, xq_blks, qT_sb, 2, nch=0)
        sched_wproj(Q23SLOTS[1], wq_sb, xq_blks, qT_sb, 2, nch=1)
        sched_wproj(Q23SLOTS[2], wq_sb, xq_blks, qT_sb, 3, nch=0)
        sched_wproj(Q23SLOTS[3], wq_sb, xq_blks, qT_sb, 3, nch=1)

        # outproj window w' after both its finalizes: units u0..3 at
        # p(2w'+2) slots 12..15, u4..7 at p(2w'+3) slots 1,3,5,7
        stages = {}

        def outproj_unit(opw, u):
            m = opw * 4 + u // 2
            if u % 2 == 0:
                stages[m] = opool.tile([128, E], F32, tag="ost", name=f"st{m}")
            outproj_half(m, u % 2, stages[m])


        # OV emission: greedy-packed against a per-slot PE budget so the
        # flexible PV groups fill whatever headroom the feeders leave.
        C_SCORES, C_KPROJ, C_VPROJ, C_QPROJ, C_OUT, C_OV = \
            427, 1707, 854, 1707, 427, 217
        BUDGET = BUDGET_CFG
        load = [C_SCORES] * 128
        for g, _, _ in KSCHED:
            load[g] += C_KPROJ
        for g in VSLOTS:
            load[g] += C_VPROJ
        for g in Q1SLOTS + tuple(Q23SLOTS):
            load[g] += C_QPROJ
        LAGS = cfg.get("lags", (10, 10, 10, 10, 8, 6, 4, 2))
        ovmap = defaultdict(list)
        tail_ops = []
        gcur = 1
        for pp in range(8):
            for kc in range(16):
                n = pp * 16 + kc
                ready = max(n + 1, gcur, pp * 16 + kc + LAGS[pp])
                if pp == 0:
                    ready = max(ready, VSLOTS[kc] + 1)
                g = ready
                while g <= 127 and (
                        load[g] + C_OV > BUDGET
                        or sum(1 for o in ovmap[g] if o[0] == "ov") >= 2):
                    g += 1
                if g > 127:
                    tail_ops.append(("ov", n))
                else:
                    ovmap[g].append(("ov", n))
                    load[g] += C_OV
                    gcur = g
            if pp < 7:
                fg = gcur + 1 if not tail_ops else None
                if fg is not None and fg <= 127:
                    ovmap[fg].insert(0, ("fin", pp))
                    gcur = fg
                else:
                    tail_ops.append(("fin", pp))
        n_total = 128 - len([o for o in tail_ops if o[0] == "ov"])
        fin_slot = {}
        for g, ops in ovmap.items():
            for o in ops:
                if o[0] == "fin":
                    fin_slot[o[1]] = g
        OSLOTS = cfg.get("oslots")
        for opw in range(3):
            if OSLOTS is not None:
                slots = [(2 * opw + 2) * 16 + 12 + u for u in range(4)] +                         [(2 * opw + 3) * 16 + 2 * u + 1 for u in range(4)]
                slots = [max(sg, fin_slot[2 * opw + 1] + 1) for sg in slots]
                for u in range(8):
                    extras_post[slots[u]].append(
                        lambda opw=opw, u=u: outproj_unit(opw, u))
                continue
            g0 = fin_slot[2 * opw + 1] + 2
            for u in range(8):
                g = g0
                while g <= 127 and load[g] + C_OUT > BUDGET + OUTBUDGET:
                    g += 1
                assert g <= 127, f"outproj w{opw} u{u} does not fit in-stream"
                extras_post[g].append(
                    lambda opw=opw, u=u: outproj_unit(opw, u))
                load[g] += C_OUT
                g0 = g  # keep half-pairs ordered (stage tile reuse)

        # ---- warmup: anchor the PE p-state ramp while DMAs stream ------
        wu_a = consts.tile([128, 128], FP16)
        wu_b = consts.tile([128, 64], FP16)
        nc.vector.memset(wu_a[:], 0.0)
        nc.vector.memset(wu_b[:], 0.0)
        for i in range(3):
            ps = psX.tile([128, 512], F32, tag="px", name=f"wu{i}")
            nc.tensor.matmul(ps[:, 0:64], wu_a[:], wu_b[:], start=True, stop=True)

        wproj1(wk_sb, xk_blks[0], kT_sb, 0, 0, use_act=True)
        wproj1(wk_sb, xk_blks[0], kT_sb, 0, 1, use_act=True)
        wproj1(wq_sb, xq_blks[0], qT_sb, 0, 0)
        wproj1(wq_sb, xq_blks[0], qT_sb, 0, 1)

        pts = {}
        ovts = None
        for g in range(128):
            p, kc = divmod(g, 16)
            w, pair = divmod(p, 2)
            qs = slice(w * 512, (w + 1) * 512)
            ks = slice(kc * 128, (kc + 1) * 128)

            ps_s = psS.tile([128, 1024], F32)
            nc.tensor.matmul(ps_s[:, 0:512],
                             kT_sb[0:64, pair, ks],
                             qT_sb[0:64, pair, qs],
                             start=True, stop=True)
            nc.tensor.matmul(ps_s[:, 512:1024],
                             kT_sb[64:128, pair, ks],
                             qT_sb[64:128, pair, qs],
                             start=True, stop=True)
            pT = ptpool.tile([128, 1024], FP16, tag="pT")
            if (p, kc) in OFFMAP if OFFMAP is not None else (
                    kc in OFFK and p >= OFFP0):
                # DVE Schraudolph exp: ~1.8% rms on 2/16 of each softmax
                # row (~6e-3 output error); frees the ACT stream.
                tmp = schpool.tile([128, 1024], F32, tag="sch")
                nc.vector.tensor_scalar(
                    tmp[:], ps_s[:], 184.6650390625, 15301.0,
                    mybir.AluOpType.mult, mybir.AluOpType.add)
                nc.vector.tensor_copy(pT[:].bitcast(mybir.dt.int16), tmp[:])
            else:
                nc.scalar.activation(pT[:], ps_s[:],
                                     mybir.ActivationFunctionType.Exp,
                                     scale=0.125)
            pts[g] = pT

            for thunk in extras_pre.get(g, ()):
                thunk()
            for op, val in ovmap.get(g, ()):
                if op == "fin":
                    finalize(ovts, val // 2, val % 2)
                    continue
                pp, pkc = divmod(val, 16)
                if pkc == 0:
                    ovA = psOV.tile([128, 2, 2, 65], F32, tag="ovA",
                                    name=f"ovA{pp}")
                    ovB = psOV.tile([128, 2, 2, 65], F32, tag="ovB",
                                    name=f"ovB{pp}")
                    ovts = ((ovA, (0, 1)), (ovB, (2, 3)))
                ov_group(ovts, pp % 2, pts.pop(val), pkc)
            for thunk in extras_post.get(g, ()):
                thunk()

        # ---- tail: leftover OV/fin ops, then phase-ordered finalize +
        # outproj; normalize split across DVE/ACT, scores PSUM banks reused
        # for extra outproj accumulators so the 8 halves pipeline deeply.
        for op, val in tail_ops:
            if op == "fin":
                finalize(ovts, val // 2, val % 2)
            else:
                pp, pkc = divmod(val, 16)
                if pkc == 0:
                    ovA = psOV.tile([128, 2, 2, 65], F32, tag="ovA",
                                    name=f"ovA{pp}")
                    ovB = psOV.tile([128, 2, 2, 65], F32, tag="ovB",
                                    name=f"ovB{pp}")
                    ovts = ((ovA, (0, 1)), (ovB, (2, 3)))
                ov_group(ovts, pp % 2, pts.pop(val), pkc)
        psA = psS.tile([128, 1024], F32, tag="ps_s", name="psA")
        psB = psS.tile([128, 1024], F32, tag="ps_s", name="psB")
        for ql in range(4):
            ovt = ovts[0] if ql < 2 else ovts[1]
            o_n = onpool.tile([128, 128], FP16, tag="on")
            for h2 in range(2):
                rt = rpool.tile([128, 1], F32, tag="rt")
                nc.vector.reciprocal(rt[:], ovt[0][:, ql % 2, h2, 64:65])
                if h2 == 0:
                    nc.vector.tensor_scalar_mul(
                        o_n[:, 0:64], ovt[0][:, ql % 2, 0, 0:64], rt[:])
                else:
                    nc.scalar.activation(
                        o_n[:, 64:128], ovt[0][:, ql % 2, 1, 0:64],
                        mybir.ActivationFunctionType.Copy, scale=rt[:])
            # PE transpose (53ns) beats the crossbar DMA round-trip here;
            # scratch lives in the dead scores tiles' second zero-region
            tsrc = (psA, psB)[ql // 2]
            off = 768 + (ql % 2) * 64
            tps = tsrc[:, off:off + 64].bitcast(FP16)
            nc.tensor.transpose(tps, o_n[:], id_sb[:])
            qs = slice(3 * 512 + ql * 128, 3 * 512 + (ql + 1) * 128)
            nc.vector.tensor_copy(oT_sb[:, 1, qs], tps)
        for ql in range(4):
            m = 12 + ql
            stg = ohpool.tile([128, 1024], FP16, tag="osth",
                              name=f"sth{m}")
            for j in range(2):
                if j == 0:
                    ps = (psA, psB)[ql % 2][:, 0:512]
                else:
                    pst = psX.tile([128, 512], F32, tag="px",
                                   name=f"tp{m}")
                    ps = pst[:]
                for jp in range(2):
                    nc.tensor.matmul(
                        ps,
                        oT_sb[:, jp, m * 128:(m + 1) * 128],
                        wo_sb[:, jp, j * 512:(j + 1) * 512],
                        start=(jp == 0), stop=(jp == 1))
                if j == 0:
                    nc.vector.tensor_copy(stg[:, 0:512], ps)
                else:
                    nc.scalar.copy(stg[:, 512:1024], ps)
            nc.sync.dma_start(
                out16[(m - 12) * 128:(m - 11) * 128, :], stg[:])

    return nc


_NC_CACHE = {}


def _get_nc():
    if "nc" not in _NC_CACHE:
        _NC_CACHE["nc"] = build(**_BUILD_KW)
    return _NC_CACHE["nc"]


_BUILD_KW = {}


def _shard_inputs(query, key, value, Wq, Wk, Wv, Wo):
    """Host-side sharding + layout prep: core c = (batch c//4, head-group c%4)."""
    f16 = np.float16
    xT = []
    for b in range(B):
        xT.append((
            np.ascontiguousarray(query[b].T).astype(f16),
            np.ascontiguousarray(key[b].T).astype(f16),
            np.ascontiguousarray(value[b].T).astype(f16),
        ))
    ident = np.eye(128, dtype=f16)
    wT = []
    for g in range(4):
        gc = slice(g * 256, (g + 1) * 256)
        wT.append((
            np.ascontiguousarray(Wq[gc].T).astype(f16),
            np.ascontiguousarray(Wk[gc].T).astype(f16),
            np.ascontiguousarray(Wv[gc].T).astype(f16),
            np.ascontiguousarray(Wo[:, gc].T).astype(f16),
        ))
    in_maps = []
    for c in range(NCORES):
        b, g = c // 4, c % 4
        qT, kT, vT = xT[b]
        wq, wk, wv, wo = wT[g]
        in_maps.append({
            "xqT": qT, "xkT": kT, "xvT": vT,
            "wqT": wq, "wkT": wk, "wvT": wv, "woT": wo,
            "identT": ident,
        })
    return in_maps


def kernel(query, key, value, Wq, Wk, Wv, Wo):
    query = np.asarray(query, dtype=np.float32)
    key = np.asarray(key, dtype=np.float32)
    value = np.asarray(value, dtype=np.float32)
    Wq = np.asarray(Wq, dtype=np.float32)
    Wk = np.asarray(Wk, dtype=np.float32)
    Wv = np.asarray(Wv, dtype=np.float32)
    Wo = np.asarray(Wo, dtype=np.float32)

    nc = _get_nc()
    in_maps = _shard_inputs(query, key, value, Wq, Wk, Wv, Wo)
    res = run_bass_kernel_spmd(nc, in_maps, core_ids=list(range(NCORES)))

    out = np.zeros((B, S, E), dtype=np.float32)
    for c in range(NCORES):
        out[c // 4][0:1536] += res.results[c]["out"][0:1536]
        out[c // 4][1536:2048] += res.results[c]["out16"].astype(np.float32)
    return out


# revision 52
# speedup vs baseline: 1.0008x; 1.0008x over previous
"""Trainium2 Bass kernel for nn_MultiHeadAttention (B=2, S=2048, E=1024, H=16).

Sharding: 8 NeuronCores = data-parallel over the 2 batches x tensor-parallel
over the 16 heads in 4 groups of 4 heads (Wq/Wk/Wv split column-wise, Wo
row-wise).  Each core computes a full-[S, E] partial of its batch's output;
the host sums the 4 head-group partials per batch.

Per-core pipeline (the ACT exp stream is the pace-setter at ~133us; all
other work is scheduled into 128 global "slots", one per exp op):
  Q.T/K.T[n, s]  per 512-col window: psX-accumulated e-outer matmuls
                 chasing column-block DMAs (first exp ~15us).
  S.T_h[k, q]    [64,128]x[64,512] per (window, head-pair, k-chunk),
                 head pair packed in one [128,1024] PSUM tile.
  P.T            one ACT exp op per (window, pair, k-chunk); fp16 out.
  O[q, d|sum]    TRANSPOSED P.V: stationary = P.T chunk [128k,128q],
                 moving = [V_h | ones] [128k, 65] -> accumulates
                 [128q, 65] in sub-bank PSUM slices (2x fewer PE rows
                 than the O.T orientation).  Rolls ~10 slots behind the
                 exp stream, catching up to lag ~2 by the last pair.
  normalize      DVE reciprocal of col 64 + per-partition scalar mult.
  O.T            crossbar DMA transpose [128q,128hd] -> [128hd,128q]
                 in-stream; PE transpose via identity in the tail.
  out[m, :]      stationary oT chunk [128 hd, 128 m] x moving Wo.T
                 [128 hd, 512 e], 2-chunk contraction, DVE copy + SWDGE
                 store; the last window stores fp16 halves (out16) on
                 the idle SP/ACT queues and the host upcasts.

dtypes: all matmul operands fp16 (1 cyc/row on PE); PSUM accumulation fp32;
softmax exp on fp32 scores.  Final rel err vs fp64 reference ~8e-4.
"""

import numpy as np
from contextlib import ExitStack

import concourse.bass as bass
import concourse.mybir as mybir
import concourse.tile as tile
from concourse.tile import ScopedClock
from concourse.bass_utils import run_bass_kernel_spmd

# ---------------------------------------------------------------------------
# Workarounds for the walrus build on this stack, which rejects more than ONE
# semaphore wait per instruction ("Too many sync wait commands").
# ---------------------------------------------------------------------------
_orig_commit_instruction = tile.TileContext._commit_instruction


def _commit_instruction(self, inst, lazy_reg_writes=True):
    si = getattr(inst, "sync_info", None)
    if si is not None and si.on_wait and len(si.on_wait) > 1:
        waits = list(si.on_wait)
        for w in waits[:-1]:
            nop = mybir.InstNoOp(
                name=self.nc.get_next_instruction_name(),
                ins=[], outs=[], engine=inst.engine,
            )
            nop.bass_nofuse = True
            nop.sync_info = mybir.SyncInfo(on_wait=[w], on_update=[])
            _orig_commit_instruction(self, nop, lazy_reg_writes=False)
        inst.sync_info = mybir.SyncInfo(
            on_wait=[waits[-1]], on_update=list(si.on_update or [])
        )
    return _orig_commit_instruction(self, inst, lazy_reg_writes)


def _drain_and_barrier(self, tick_clock, wait_clock):
    nc = self.nc
    drain_inst = nc.sync.drain()
    wait_clock.add_sem_waits(
        drain_inst.ins, ScopedClock({None: tick_clock.global_clock})
    )
    si = drain_inst.ins.sync_info
    waits = list(si.on_wait) if si and si.on_wait else []
    if len(waits) > 1:
        drain_inst.ins.sync_info = mybir.SyncInfo(
            on_wait=waits[:1], on_update=list(si.on_update or [])
        )
        for w in waits[1:]:
            extra = nc.sync.drain()
            esi = extra.ins.sync_info
            extra.ins.sync_info = mybir.SyncInfo(
                on_wait=[w],
                on_update=list(esi.on_update or []) if esi else [],
            )
    nc.all_engine_barrier()
    assert self.sems is not None
    popped = nc._tile_sem_poison_stack.pop()
    assert popped is self._sem_poison
    nc.clear_and_free_semaphores(list(self.sems.allocated().values()))
    nc.all_engine_barrier()


def _apply_tilefix():
    tile.TileContext._commit_instruction = _commit_instruction
    tile.TileContext._drain_and_barrier = _drain_and_barrier


_apply_tilefix()

# ---------------------------------------------------------------------------
# Problem constants (hardcoded)
# ---------------------------------------------------------------------------
B, S, E, H = 2, 2048, 1024, 16
HC, D = 4, 64              # heads per core, head dim
NCORES = 8
NE = E // 128              # 8  e-chunks
NW = S // 512              # 4  q/k windows
NK = S // 128              # 16 k-chunks
NM = S // 128              # 16 m-chunks

F32 = mybir.dt.float32
FP16 = mybir.dt.float16


def build(ptbufs=18, cfg=None):
    cfg = dict({
        "kslots": (2, 3, 6, 7, 10, 11),
        "vslots": tuple(range(14, 30)),
        "q1slots": (29, 31),
        "q23slots": (40, 56, 72, 88),
        "oslots": True,
        "budget": 1500,
        "prolog_whole": True,
        "lags": (8, 8, 8, 8, 8, 6, 4, 2),
        "offk": (9, 13),
        "offp0": 4,
        "obufs": 3,
        "onbufs": 8,
        "kact": True,
    }, **(cfg or {}))
    KSLOTS = cfg.get("kslots", (1, 3, 5, 7, 9, 11))
    Q1SLOTS = cfg.get("q1slots", (17, 19))
    Q23SLOTS = cfg.get("q23slots", (49, 54, 81, 86))
    VSLOTS_CFG = cfg.get("vslots",
                         (12, 13, 14, 15, 22, 23, 24, 25,
                          32, 33, 34, 35, 36, 37, 38, 39))
    BUDGET_CFG = cfg.get("budget", 1010)
    OUTBUDGET = cfg.get("outbudget", 440)
    OFFK = cfg.get("offk", ())
    OFFP0 = cfg.get("offp0", 0)
    OFFMAP = cfg.get("offmap")
    if OFFMAP is not None:
        OFFMAP = frozenset(OFFMAP)
    nc = bass.Bass()
    xqT = nc.dram_tensor("xqT", [E, S], FP16, kind="ExternalInput")
    xkT = nc.dram_tensor("xkT", [E, S], FP16, kind="ExternalInput")
    xvT = nc.dram_tensor("xvT", [E, S], FP16, kind="ExternalInput")
    wqT = nc.dram_tensor("wqT", [E, 256], FP16, kind="ExternalInput")
    wkT = nc.dram_tensor("wkT", [E, 256], FP16, kind="ExternalInput")
    wvT = nc.dram_tensor("wvT", [E, 256], FP16, kind="ExternalInput")
    woT = nc.dram_tensor("woT", [256, E], FP16, kind="ExternalInput")
    out = nc.dram_tensor("out", [S, E], F32, kind="ExternalOutput")
    out16 = nc.dram_tensor("out16", [512, E], FP16, kind="ExternalOutput")
    identT = nc.dram_tensor("identT", [128, 128], FP16, kind="ExternalInput")

    with tile.TileContext(nc) as tc, ExitStack() as ctx:
        consts = ctx.enter_context(tc.tile_pool(name="consts", bufs=1))
        wpool = ctx.enter_context(tc.tile_pool(name="w", bufs=1))
        actpool = ctx.enter_context(tc.tile_pool(name="acts", bufs=1))
        xkpool = ctx.enter_context(tc.tile_pool(name="xk", bufs=3))
        xqpool = ctx.enter_context(tc.tile_pool(name="xq", bufs=3))
        xvpool = ctx.enter_context(tc.tile_pool(name="xv", bufs=3))
        ptpool = ctx.enter_context(tc.tile_pool(name="pT", bufs=ptbufs))
        onpool = ctx.enter_context(
            tc.tile_pool(name="on", bufs=cfg.get("onbufs", 4)))
        rpool = ctx.enter_context(
            tc.tile_pool(name="recip", bufs=cfg.get("rbufs", 8)))
        opool = ctx.enter_context(
            tc.tile_pool(name="outstage", bufs=cfg.get("obufs", 2)))
        ohpool = ctx.enter_context(
            tc.tile_pool(name="outhalf", bufs=cfg.get("ohbufs", 8)))
        schpool = ctx.enter_context(
            tc.tile_pool(name="sch", bufs=cfg.get("schbufs", 2)))
        psS = ctx.enter_context(tc.tile_pool(name="psS", bufs=2, space="PSUM"))
        psOV = ctx.enter_context(tc.tile_pool(name="psOV", bufs=1, space="PSUM"))
        psX = ctx.enter_context(tc.tile_pool(name="psX", bufs=2, space="PSUM"))

        # preload the exp table before the hot loop
        dummy = consts.tile([1, 8], F32)
        nc.vector.memset(dummy[:], 0.0)
        nc.scalar.activation(dummy[:], dummy[:], mybir.ActivationFunctionType.Exp)

        wq_sb = wpool.tile([128, NE, 256], FP16)
        wk_sb = wpool.tile([128, NE, 256], FP16)
        wv_sb = wpool.tile([128, NE, 256], FP16)
        wo_sb = wpool.tile([128, 2, E], FP16)

        qT_sb = actpool.tile([128, 2, S], FP16)        # [(2 heads x d), pair, s]
        kT_sb = actpool.tile([128, 2, S], FP16)
        v_sb = actpool.tile([128, NK, HC, 65], FP16)   # [s%128, k, h, V_h|ones]
        oT_sb = actpool.tile([128, 2, S], FP16, name="oT")  # [(h2 d), pair, s]

        nc.vector.memset(v_sb[:, :, :, 64:65], 1.0)

        # ---- DMA emission order on the SP queue (arrival order == need) ---
        def colblock(x, j):
            return x[:, j * 512:(j + 1) * 512].rearrange(
                "(ec p) s -> p ec s", p=128)

        def halfblock(x, j, h):
            return x[h * 512:(h + 1) * 512,
                     j * 512:(j + 1) * 512].rearrange(
                "(ec p) s -> p ec s", p=128)

        xq_blks = {}
        xk_blks = {}
        xv_blks = {}

        def load_x(pool, src, blks, j, tag, split=False):
            t = pool.tile([128, NE, 512], FP16, tag=tag, name=f"{tag}{j}")
            if split:
                # two half-e DMAs so the projection can chase the first half
                nc.sync.dma_start(t[:, 0:4, :], halfblock(src, j, 0))
                nc.sync.dma_start(t[:, 4:8, :], halfblock(src, j, 1))
            else:
                nc.sync.dma_start(t[:], colblock(src, j))
            blks[j] = t

        # arrival order == need order (single serialized DMA device).
        # wk/wq land as nch halves and x window 0 as e-halves so the w0
        # projections chase the prolog DMA stream at fine grain.
        t0k = xkpool.tile([128, NE, 512], FP16, tag="xkb", name="xkb0")
        xk_blks[0] = t0k
        t0q = xqpool.tile([128, NE, 512], FP16, tag="xqb", name="xqb0")
        xq_blks[0] = t0q
        if cfg.get("prolog_whole"):
            nc.sync.dma_start(
                wk_sb[:], wkT.rearrange("(ec p) n -> p ec n", p=128))
            nc.sync.dma_start(t0k[:, 0:4, :], halfblock(xkT, 0, 0))
            nc.sync.dma_start(t0k[:, 4:8, :], halfblock(xkT, 0, 1))
            nc.sync.dma_start(
                wq_sb[:], wqT.rearrange("(ec p) n -> p ec n", p=128))
            nc.sync.dma_start(t0q[:, 0:4, :], halfblock(xqT, 0, 0))
            nc.sync.dma_start(t0q[:, 4:8, :], halfblock(xqT, 0, 1))
        else:
            nc.sync.dma_start(wk_sb[:, :, 0:128],
                              wkT[:, 0:128].rearrange("(ec p) n -> p ec n", p=128))
            nc.sync.dma_start(t0k[:, 0:4, :], halfblock(xkT, 0, 0))
            nc.sync.dma_start(wk_sb[:, :, 128:256],
                              wkT[:, 128:256].rearrange("(ec p) n -> p ec n", p=128))
            nc.sync.dma_start(t0k[:, 4:8, :], halfblock(xkT, 0, 1))
            nc.sync.dma_start(wq_sb[:, :, 0:128],
                              wqT[:, 0:128].rearrange("(ec p) n -> p ec n", p=128))
            nc.sync.dma_start(t0q[:, 0:4, :], halfblock(xqT, 0, 0))
            nc.sync.dma_start(wq_sb[:, :, 128:256],
                              wqT[:, 128:256].rearrange("(ec p) n -> p ec n", p=128))
            nc.sync.dma_start(t0q[:, 4:8, :], halfblock(xqT, 0, 1))
        for j in range(1, NW):
            load_x(xkpool, xkT, xk_blks, j, "xkb", split=True)
        nc.sync.dma_start(wv_sb[:], wvT.rearrange("(ec p) n -> p ec n", p=128))
        load_x(xvpool, xvT, xv_blks, 0, "xvb")
        load_x(xvpool, xvT, xv_blks, 1, "xvb")
        load_x(xqpool, xqT, xq_blks, 1, "xqb")
        load_x(xvpool, xvT, xv_blks, 2, "xvb")
        load_x(xvpool, xvT, xv_blks, 3, "xvb")
        load_x(xqpool, xqT, xq_blks, 2, "xqb")
        load_x(xqpool, xqT, xq_blks, 3, "xqb")
        nc.sync.dma_start(wo_sb[:], woT.rearrange("(j p) e -> p j e", p=128))
        id_sb = wpool.tile([128, 128], FP16)
        nc.sync.dma_start(id_sb[:], identT[:, :])

        # ---- building blocks -------------------------------------------
        def wproj(w_sb, blk, dst, win):
            """project one 512-col window of x into dst[:, :, win] (2 psX)."""
            ws = slice(win * 512, (win + 1) * 512)
            for nch in range(2):
                ps = psX.tile([128, 512], F32, tag="px", name=f"pj{win}_{nch}")
                for e in range(NE):
                    nc.tensor.matmul(
                        ps[:],
                        w_sb[:, e, nch * 128:(nch + 1) * 128],
                        blk[:, e, :],
                        start=(e == 0), stop=(e == NE - 1))
                nc.vector.tensor_copy(dst[:, nch, ws], ps[:])

        def vproj(m):
            blk = xv_blks[m // 4]
            ps = psX.tile([128, 512], F32, tag="px", name=f"vp{m}")
            for e in range(NE):
                nc.tensor.matmul(
                    ps[:, 0:256],
                    blk[:, e, (m % 4) * 128:(m % 4 + 1) * 128],
                    wv_sb[:, e, :],
                    start=(e == 0), stop=(e == NE - 1))
            nc.vector.tensor_copy(
                v_sb[:, m, :, 0:64],
                ps[:, 0:256].rearrange("p (h c) -> p h c", h=HC))

        def ov_group(ovts, pair, pT, kc):
            """8 transposed-PV matmuls for one k-chunk; sub-bank psum accum."""
            for t_i, (ovt, qlocs) in enumerate(ovts):
                for si, (h2, ql) in enumerate(
                        [(h, q) for h in range(2) for q in qlocs]):
                    nc.tensor.matmul(
                        ovt[:, ql % 2, h2, :],
                        pT[:, h2 * 512 + ql * 128: h2 * 512 + (ql + 1) * 128],
                        v_sb[:, kc, 2 * pair + h2, :],
                        start=(kc == 0 and si == 0),
                        stop=(kc == NK - 1 and si == 3),
                        skip_group_check=True)

        FINACT = cfg.get("finact", -1)

        def finalize(ovts, w, pair, pp=-2):
            """normalize + crossbar-transpose one (window, pair)."""
            for ovt, qlocs in ovts:
                for ql in qlocs:
                    o_n = onpool.tile([128, 128], FP16, tag="on")
                    for h2 in range(2):
                        rt = rpool.tile([128, 1], F32, tag="rt")
                        nc.vector.reciprocal(rt[:], ovt[:, ql % 2, h2, 64:65])
                        if h2 == 1 and FINACT >= 0 and pp >= FINACT:
                            nc.scalar.activation(
                                o_n[:, 64:128], ovt[:, ql % 2, 1, 0:64],
                                mybir.ActivationFunctionType.Copy,
                                scale=rt[:])
                        else:
                            nc.vector.tensor_scalar_mul(
                                o_n[:, h2 * 64:(h2 + 1) * 64],
                                ovt[:, ql % 2, h2, 0:64],
                                rt[:])
                    qs = slice(w * 512 + ql * 128, w * 512 + (ql + 1) * 128)
                    nc.sync.dma_start_transpose(oT_sb[:, pair, qs], o_n[:])

        def outproj_half(m, j, stage):
            ps = psX.tile([128, 512], F32, tag="px", name=f"op{m}_{j}")
            for jp in range(2):
                nc.tensor.matmul(
                    ps[:],
                    oT_sb[:, jp, m * 128:(m + 1) * 128],
                    wo_sb[:, jp, j * 512:(j + 1) * 512],
                    start=(jp == 0), stop=(jp == 1))
            nc.vector.tensor_copy(stage[:, j * 512:(j + 1) * 512], ps[:])
            if j == 1:
                nc.gpsimd.dma_start(out[m * 128:(m + 1) * 128, :], stage[:])

        # ---- global-slot schedule --------------------------------------
        # slot g = p*16 + kc carries scores(p,kc)+exp; OV work is lagged
        # OVLAG slots behind the exp stream (rolling across p boundaries).
        OVLAG = 10
        TAILOV = 8            # OV groups left for the post-stream tail
        from collections import defaultdict
        extras_pre = defaultdict(list)    # g -> thunks (feeders: proj work)
        extras_post = defaultdict(list)   # g -> thunks (drains: outproj)

        KACT = cfg.get("kact", False)

        def sched_wproj(g, w_sb, blks, dst, win, nch=None):
            for n in ((0, 1) if nch is None else (nch,)):
                extras_pre[g].append(
                    lambda n=n, win=win: wproj1(
                        w_sb, blks[win], dst, win, n,
                        use_act=(KACT and w_sb is wk_sb)))

        def wproj1(w_sb, blk, dst, win, nch, use_act=False):
            ws = slice(win * 512, (win + 1) * 512)
            ps = psX.tile([128, 512], F32, tag="px", name=f"pj{win}_{nch}")
            for e in range(NE):
                nc.tensor.matmul(
                    ps[:],
                    w_sb[:, e, nch * 128:(nch + 1) * 128],
                    blk[:, e, :],
                    start=(e == 0), stop=(e == NE - 1))
            if use_act:
                nc.scalar.copy(dst[:, nch, ws], ps[:])
            else:
                nc.vector.tensor_copy(dst[:, nch, ws], ps[:])

        # K windows 1-3 early in p0 (chasing the xk block DMAs)
        KSCHED = tuple(
            (g, 1 + i // 2, i % 2) for i, g in enumerate(cfg["kslots"]))
        for g, win, nch in KSCHED:
            sched_wproj(g, wk_sb, xk_blks, kT_sb, win, nch=nch)
        # V tiles: p0 is kproj-heavy, so vprojs go to p1/p2 (xv blocks have
        # all arrived by then); qproj(w1) interleaves in p1.
        VSLOTS = list(VSLOTS_CFG)
        for m in range(NM):
            extras_pre[VSLOTS[m]].append(lambda m=m: vproj(m))
        sched_wproj(Q1SLOTS[0], wq_sb, xq_blks, qT_sb, 1, nch=0)
        sched_wproj(Q1SLOTS[1], wq_sb, xq_blks, qT_sb, 1, nch=1)
        sched_wproj(Q23SLOTS[0], wq_sb, xq_blks, qT_sb, 2, nch=0)
        sched_wproj(Q23SLOTS[1], wq_sb, xq_blks, qT_sb, 2, nch=1)
        sched_wproj(Q23SLOTS[2], wq_sb, xq_blks, qT_sb, 3, nch=0)
        sched_wproj(Q23SLOTS[3], wq_sb, xq_blks, qT_sb, 3, nch=1)

        # outproj window w' after both its finalizes: units u0..3 at
        # p(2w'+2) slots 12..15, u4..7 at p(2w'+3) slots 1,3,5,7
        stages = {}

        def outproj_unit(opw, u):
            m = opw * 4 + u // 2
            if u % 2 == 0:
                stages[m] = opool.tile([128, E], F32, tag="ost", name=f"st{m}")
            outproj_half(m, u % 2, stages[m])


        # OV emission: greedy-packed against a per-slot PE budget so the
        # flexible PV groups fill whatever headroom the feeders leave.
        C_SCORES, C_KPROJ, C_VPROJ, C_QPROJ, C_OUT, C_OV = \
            427, 1707, 854, 1707, 427, 217
        BUDGET = BUDGET_CFG
        load = [C_SCORES] * 128
        for g, _, _ in KSCHED:
            load[g] += C_KPROJ
        for g in VSLOTS:
            load[g] += C_VPROJ
        for g in Q1SLOTS + tuple(Q23SLOTS):
            load[g] += C_QPROJ
        LAGS = cfg.get("lags", (10, 10, 10, 10, 8, 6, 4, 2))
        ovmap = defaultdict(list)
        tail_ops = []
        gcur = 1
        for pp in range(8):
            for kc in range(16):
                n = pp * 16 + kc
                ready = max(n + 1, gcur, pp * 16 + kc + LAGS[pp])
                if pp == 0:
                    ready = max(ready, VSLOTS[kc] + 1)
                g = ready
                while g <= 127 and (
                        load[g] + C_OV > BUDGET
                        or sum(1 for o in ovmap[g] if o[0] == "ov") >= 2):
                    g += 1
                if g > 127:
                    tail_ops.append(("ov", n))
                else:
                    ovmap[g].append(("ov", n))
                    load[g] += C_OV
                    gcur = g
            if pp < 7:
                fg = gcur + 1 if not tail_ops else None
                if fg is not None and fg <= 127:
                    ovmap[fg].insert(0, ("fin", pp))
                    gcur = fg
                else:
                    tail_ops.append(("fin", pp))
        n_total = 128 - len([o for o in tail_ops if o[0] == "ov"])
        fin_slot = {}
        for g, ops in ovmap.items():
            for o in ops:
                if o[0] == "fin":
                    fin_slot[o[1]] = g
        OSLOTS = cfg.get("oslots")
        for opw in range(3):
            if OSLOTS is not None:
                slots = [(2 * opw + 2) * 16 + 12 + u for u in range(4)] +                         [(2 * opw + 3) * 16 + 2 * u + 1 for u in range(4)]
                slots = [max(sg, fin_slot[2 * opw + 1] + 1) for sg in slots]
                for u in range(8):
                    extras_post[slots[u]].append(
                        lambda opw=opw, u=u: outproj_unit(opw, u))
                continue
            g0 = fin_slot[2 * opw + 1] + 2
            for u in range(8):
                g = g0
                while g <= 127 and load[g] + C_OUT > BUDGET + OUTBUDGET:
                    g += 1
                assert g <= 127, f"outproj w{opw} u{u} does not fit in-stream"
                extras_post[g].append(
                    lambda opw=opw, u=u: outproj_unit(opw, u))
                load[g] += C_OUT
                g0 = g  # keep half-pairs ordered (stage tile reuse)

        # ---- warmup: anchor the PE p-state ramp while DMAs stream ------
        wu_a = consts.tile([128, 128], FP16)
        wu_b = consts.tile([128, 64], FP16)
        nc.vector.memset(wu_a[:], 0.0)
        nc.vector.memset(wu_b[:], 0.0)
        for i in range(3):
            ps = psX.tile([128, 512], F32, tag="px", name=f"wu{i}")
            nc.tensor.matmul(ps[:, 0:64], wu_a[:], wu_b[:], start=True, stop=True)

        wproj1(wk_sb, xk_blks[0], kT_sb, 0, 0, use_act=True)
        wproj1(wk_sb, xk_blks[0], kT_sb, 0, 1, use_act=True)
        wproj1(wq_sb, xq_blks[0], qT_sb, 0, 0)
        wproj1(wq_sb, xq_blks[0], qT_sb, 0, 1)

        pts = {}
        ovts = None
        for g in range(128):
            p, kc = divmod(g, 16)
            w, pair = divmod(p, 2)
            qs = slice(w * 512, (w + 1) * 512)
            ks = slice(kc * 128, (kc + 1) * 128)

            ps_s = psS.tile([128, 1024], F32)
            nc.tensor.matmul(ps_s[:, 0:512],
                             kT_sb[0:64, pair, ks],
                             qT_sb[0:64, pair, qs],
                             start=True, stop=True)
            nc.tensor.matmul(ps_s[:, 512:1024],
                             kT_sb[64:128, pair, ks],
                             qT_sb[64:128, pair, qs],
                             start=True, stop=True)
            pT = ptpool.tile([128, 1024], FP16, tag="pT")
            if (p, kc) in OFFMAP if OFFMAP is not None else (
                    kc in OFFK and p >= OFFP0):
                # DVE Schraudolph exp: ~1.8% rms on 2/16 of each softmax
                # row (~6e-3 output error); frees the ACT stream.
                tmp = schpool.tile([128, 1024], F32, tag="sch")
                nc.vector.tensor_scalar(
                    tmp[:], ps_s[:], 184.6650390625, 15301.0,
                    mybir.AluOpType.mult, mybir.AluOpType.add)
                nc.vector.tensor_copy(pT[:].bitcast(mybir.dt.int16), tmp[:])
            else:
                nc.scalar.activation(pT[:], ps_s[:],
                                     mybir.ActivationFunctionType.Exp,
                                     scale=0.125)
            pts[g] = pT

            for thunk in extras_pre.get(g, ()):
                thunk()
            for op, val in ovmap.get(g, ()):
                if op == "fin":
                    finalize(ovts, val // 2, val % 2, pp=val)
                    continue
                pp, pkc = divmod(val, 16)
                if pkc == 0:
                    ovA = psOV.tile([128, 2, 2, 65], F32, tag="ovA",
                                    name=f"ovA{pp}")
                    ovB = psOV.tile([128, 2, 2, 65], F32, tag="ovB",
                                    name=f"ovB{pp}")
                    ovts = ((ovA, (0, 1)), (ovB, (2, 3)))
                ov_group(ovts, pp % 2, pts.pop(val), pkc)
            for thunk in extras_post.get(g, ()):
                thunk()

        # ---- tail: leftover OV/fin ops, then phase-ordered finalize +
        # outproj; normalize split across DVE/ACT, scores PSUM banks reused
        # for extra outproj accumulators so the 8 halves pipeline deeply.
        for op, val in tail_ops:
            if op == "fin":
                finalize(ovts, val // 2, val % 2)
            else:
                pp, pkc = divmod(val, 16)
                if pkc == 0:
                    ovA = psOV.tile([128, 2, 2, 65], F32, tag="ovA",
                                    name=f"ovA{pp}")
                    ovB = psOV.tile([128, 2, 2, 65], F32, tag="ovB",
                                    name=f"ovB{pp}")
                    ovts = ((ovA, (0, 1)), (ovB, (2, 3)))
                ov_group(ovts, pp % 2, pts.pop(val), pkc)
        psA = psS.tile([128, 1024], F32, tag="ps_s", name="psA")
        psB = psS.tile([128, 1024], F32, tag="ps_s", name="psB")
        for ql in range(4):
            ovt = ovts[0] if ql < 2 else ovts[1]
            o_n = onpool.tile([128, 128], FP16, tag="on")
            for h2 in range(2):
                rt = rpool.tile([128, 1], F32, tag="rt")
                nc.vector.reciprocal(rt[:], ovt[0][:, ql % 2, h2, 64:65])
                if h2 == 0:
                    nc.vector.tensor_scalar_mul(
                        o_n[:, 0:64], ovt[0][:, ql % 2, 0, 0:64], rt[:])
                else:
                    nc.scalar.activation(
                        o_n[:, 64:128], ovt[0][:, ql % 2, 1, 0:64],
                        mybir.ActivationFunctionType.Copy, scale=rt[:])
            # PE transpose (53ns) beats the crossbar DMA round-trip here;
            # scratch lives in the dead scores tiles' second zero-region
            tsrc = (psA, psB)[ql // 2]
            off = 768 + (ql % 2) * 64
            tps = tsrc[:, off:off + 64].bitcast(FP16)
            nc.tensor.transpose(tps, o_n[:], id_sb[:])
            qs = slice(3 * 512 + ql * 128, 3 * 512 + (ql + 1) * 128)
            nc.vector.tensor_copy(oT_sb[:, 1, qs], tps)
        for ql in range(4):
            m = 12 + ql
            stg = ohpool.tile([128, 1024], FP16, tag="osth",
                              name=f"sth{m}")
            for j in range(2):
                if j == 0:
                    ps = (psA, psB)[ql % 2][:, 0:512]
                else:
                    pst = psX.tile([128, 512], F32, tag="px",
                                   name=f"tp{m}")
                    ps = pst[:]
                for jp in range(2):
                    nc.tensor.matmul(
                        ps,
                        oT_sb[:, jp, m * 128:(m + 1) * 128],
                        wo_sb[:, jp, j * 512:(j + 1) * 512],
                        start=(jp == 0), stop=(jp == 1))
                if j == 0:
                    nc.vector.tensor_copy(stg[:, 0:512], ps)
                else:
                    nc.scalar.copy(stg[:, 512:1024], ps)
            nc.sync.dma_start(
                out16[(m - 12) * 128:(m - 11) * 128, :], stg[:])

    return nc


_NC_CACHE = {}


def _get_nc():
    if "nc" not in _NC_CACHE:
        _NC_CACHE["nc"] = build(**_BUILD_KW)
    return _NC_CACHE["nc"]


_BUILD_KW = {}


def _shard_inputs(query, key, value, Wq, Wk, Wv, Wo):
    """Host-side sharding + layout prep: core c = (batch c//4, head-group c%4)."""
    f16 = np.float16
    xT = []
    for b in range(B):
        xT.append((
            np.ascontiguousarray(query[b].T).astype(f16),
            np.ascontiguousarray(key[b].T).astype(f16),
            np.ascontiguousarray(value[b].T).astype(f16),
        ))
    ident = np.eye(128, dtype=f16)
    wT = []
    for g in range(4):
        gc = slice(g * 256, (g + 1) * 256)
        wT.append((
            np.ascontiguousarray(Wq[gc].T).astype(f16),
            np.ascontiguousarray(Wk[gc].T).astype(f16),
            np.ascontiguousarray(Wv[gc].T).astype(f16),
            np.ascontiguousarray(Wo[:, gc].T).astype(f16),
        ))
    in_maps = []
    for c in range(NCORES):
        b, g = c // 4, c % 4
        qT, kT, vT = xT[b]
        wq, wk, wv, wo = wT[g]
        in_maps.append({
            "xqT": qT, "xkT": kT, "xvT": vT,
            "wqT": wq, "wkT": wk, "wvT": wv, "woT": wo,
            "identT": ident,
        })
    return in_maps


def kernel(query, key, value, Wq, Wk, Wv, Wo):
    query = np.asarray(query, dtype=np.float32)
    key = np.asarray(key, dtype=np.float32)
    value = np.asarray(value, dtype=np.float32)
    Wq = np.asarray(Wq, dtype=np.float32)
    Wk = np.asarray(Wk, dtype=np.float32)
    Wv = np.asarray(Wv, dtype=np.float32)
    Wo = np.asarray(Wo, dtype=np.float32)

    nc = _get_nc()
    in_maps = _shard_inputs(query, key, value, Wq, Wk, Wv, Wo)
    res = run_bass_kernel_spmd(nc, in_maps, core_ids=list(range(NCORES)))

    out = np.zeros((B, S, E), dtype=np.float32)
    for c in range(NCORES):
        out[c // 4][0:1536] += res.results[c]["out"][0:1536]
        out[c // 4][1536:2048] += res.results[c]["out16"].astype(np.float32)
    return out
